# revision 39
# baseline (speedup 1.0000x reference)
"""Trainium2 Bass kernel for nn_Attention_28406913696361.

Architecture: B=8 batch elements -> 8 NeuronCores, pure data-parallel
(all params replicated, zero collectives). Each core computes, for its
batch element:
    k = mlp(x1), q = mlp(x2)          (shared 2-layer MLP, relu)
    qh/kh/vh = per-head projections    (H=8 heads, hd=64)
    o = softmax(qh kh^T / 8) vh        (full 2048x2048 attention)
    out = sum_h o_h @ Wo_h^T

Compute in bf16 with f32 PSUM accumulation (validated: L2 rel err ~5e-3
vs the f32 reference; gate is 2e-2). Key structural choices:

- W2 is folded into the per-head k/q projections (khT = (Wk W2) h^T), so
  MLP stage-2 and the 512-wide kq intermediate are never computed.
- Softmax denominators come free from a ones-column appended to vh: the
  PV matmul accumulates the row sum in f32 in the same instruction. No
  max-subtraction (scores/8 are bounded ~6), no vector reductions.
- exp runs on ScalarE (the bottleneck engine, ~1 elem/cycle/lane) from
  [128,1024] PSUM tiles; each tile holds one n-tile of scores for BOTH
  heads of a pair, computed by two matmuls on disjoint PE row groups
  (khT/qhT stack the pair's e' axes on partitions 0-63 / 64-127), which
  the hardware runs concurrently when issued back-to-back.
- Attention is chunk-major with the output projection woven in per
  chunk; PV iterates t-major so probs tiles release incrementally.
- Scores matmuls carry a large scheduler priority so they always beat
  queued PV matmuls: keeps ScalarE saturated.
- The output projection contracts all 128 partitions of the pair layout
  (o_hT pair-stacked e' x WoT pair-stacked e'), so one K=128 matmul per
  head-pair performs the head sum in-instruction.

Layouts (per core, partition dim first):
    x1T/x2T  [128, 2048]         d_x on partitions
    rT       [128, 4, 2048]      (d_inner, d_chunk, n)
    khT/qhT  [128, 4, 2048]      (s*64+e', head_pair, token)
    vh_ext   [128, 8, 16, 65]    (n_inner, head, n_tile, e'+ones)
    o_hT     [128, 4, 2048]      (s*64+e', pair, m)
"""

import numpy as np

N = 2048
DX = 128
D = 512
H = 8
HD = 64
P = 128
NT = N // P        # 16 token tiles
DC = D // P        # 4 feature chunks of 128
MC = N // 512      # 4 chunks of 512 tokens
NPAIR = H // 2     # 4 head pairs
NCORES = 8

_CACHE = {}


def _build_program():
    from contextlib import ExitStack

    import concourse.bass as bass  # noqa: F401
    import concourse.mybir as mybir
    import concourse.tile as tile
    from concourse import bacc
    from concourse.masks import make_identity

    fp32 = mybir.dt.float32
    bf16 = mybir.dt.bfloat16
    AF = mybir.ActivationFunctionType
    ALU = mybir.AluOpType

    nc = bacc.Bacc("TRN2")

    x1 = nc.declare_dram_parameter("x1", [N, DX], fp32, isOutput=False)
    x2 = nc.declare_dram_parameter("x2", [N, DX], fp32, isOutput=False)
    r_ = nc.declare_dram_parameter("r", [N, D], fp32, isOutput=False)
    W1 = nc.declare_dram_parameter("W1", [DX, D], fp32, isOutput=False)
    b1 = nc.declare_dram_parameter("b1", [D], fp32, isOutput=False)
    W2 = nc.declare_dram_parameter("W2", [D, D], fp32, isOutput=False)
    b2 = nc.declare_dram_parameter("b2", [D], fp32, isOutput=False)
    Wq = nc.declare_dram_parameter("Wq", [H, HD, D], fp32, isOutput=False)
    Wk = nc.declare_dram_parameter("Wk", [H, HD, D], fp32, isOutput=False)
    Wv = nc.declare_dram_parameter("Wv", [H, HD, D], fp32, isOutput=False)
    Wo = nc.declare_dram_parameter("Wo", [H, D, HD], fp32, isOutput=False)
    out = nc.declare_dram_parameter("out", [N, D], fp32, isOutput=True)

    with ExitStack() as ctx:
        tc = ctx.enter_context(tile.TileContext(nc))
        const = ctx.enter_context(tc.tile_pool(name="const", bufs=1))
        seq = ctx.enter_context(tc.tile_pool(name="seq", bufs=1))
        big = ctx.enter_context(tc.tile_pool(name="big", bufs=2))
        hpool = ctx.enter_context(tc.tile_pool(name="hpool", bufs=2))
        stage = ctx.enter_context(tc.tile_pool(name="stage", bufs=8))
        probs = ctx.enter_context(tc.tile_pool(name="probs", bufs=30))
        onorm = ctx.enter_context(tc.tile_pool(name="onorm", bufs=2))
        outp = ctx.enter_context(tc.tile_pool(name="outp", bufs=3))
        small = ctx.enter_context(tc.tile_pool(name="small", bufs=4))
        ps_mm = ctx.enter_context(tc.tile_pool(name="ps_mm", bufs=2, space="PSUM"))
        ps_sc = ctx.enter_context(tc.tile_pool(name="ps_sc", bufs=2, space="PSUM"))
        ps_po = ctx.enter_context(tc.tile_pool(name="ps_po", bufs=2, space="PSUM"))

        ident = const.tile([P, P], fp32, tag="ident")
        make_identity(nc, ident)

        # ---------------- weights ----------------
        s = stage.tile([P, D], fp32, tag="stage")
        nc.sync.dma_start(out=s, in_=W1[:, :])
        W1_bf = const.tile([P, D], bf16, tag="W1")
        nc.vector.tensor_copy(out=W1_bf, in_=s)

        w2stage = []
        for c in range(DC):
            s = stage.tile([P, D], fp32, tag="stage", name=f"w2s{c}")
            nc.sync.dma_start(out=s, in_=W2[c * P:(c + 1) * P, :])
            w2stage.append(s)

        b1_sb = const.tile([P, DC], fp32, tag="b1")
        b2_sb = const.tile([P, DC], fp32, tag="b2")
        with nc.allow_non_contiguous_dma(reason="tiny bias vectors"):
            nc.sync.dma_start(out=b1_sb, in_=b1.rearrange("(t p) -> p t", p=P))
            nc.sync.dma_start(out=b2_sb, in_=b2.rearrange("(t p) -> p t", p=P))

        # transposed qkv projection weights: [d_inner, d_chunk, (h e')]
        def load_wT(w_ap, name):
            wt = const.tile([P, DC, D], bf16, tag=name)
            flat = w_ap.rearrange("h e d -> (h e) d")
            ss = []
            for i in range(4):
                si = stage.tile([P, D], fp32, tag="stage")
                nc.sync.dma_start(out=si, in_=flat[i * P:(i + 1) * P, :])
                ss.append(si)
            for j in range(DC):
                pst = ps_mm.tile([P, 512], fp32, tag="mm")
                for i in range(4):
                    nc.tensor.transpose(
                        pst[:, i * P:(i + 1) * P], ss[i][:, j * P:(j + 1) * P], ident
                    )
                nc.vector.tensor_copy(out=wt[:, j, :], in_=pst)
            return wt

        # ---------------- input transposes ----------------
        def load_xT(x_ap):
            xt = big.tile([P, N], bf16, tag="xT")
            for g in range(4):
                ss = []
                for t in range(4):
                    si = stage.tile([P, P], fp32, tag="xstage")
                    tt = g * 4 + t
                    nc.sync.dma_start(out=si, in_=x_ap[tt * P:(tt + 1) * P, :])
                    ss.append(si)
                pst = ps_mm.tile([P, 512], fp32, tag="mm")
                for t in range(4):
                    nc.tensor.transpose(pst[:, t * P:(t + 1) * P], ss[t], ident)
                nc.vector.tensor_copy(out=xt[:, g * 512:(g + 1) * 512], in_=pst)
            return xt

        x1T = load_xT(x1)
        x2T = load_xT(x2)

        WkT = load_wT(Wk, "WkT")
        WqT = load_wT(Wq, "WqT")

        # ---- fuse W2 into the k/q head projections ----
        # khT = Wk (W2^T h^T) = (Wk W2) h^T, so precompute
        # Wfused[e, (h e')] = sum_f W2[e, f] WkT[f, (h e')] on PE (tiny), and
        # skip MLP stage-2 + the kq intermediate entirely. The b2 bias folds
        # to a per-(h,e') constant kb = Wk @ b2.
        W2T = const.tile([P, DC, D], bf16, tag="W2T")
        for j in range(DC):
            pst = ps_mm.tile([P, 512], fp32, tag="mm")
            for e in range(DC):
                nc.tensor.transpose(
                    pst[:, e * P:(e + 1) * P],
                    w2stage[e][:, j * P:(j + 1) * P], ident,
                )
            nc.vector.tensor_copy(out=W2T[:, j, :], in_=pst)

        def fuse_w2(wT, name):
            wf = const.tile([P, DC, D], bf16, tag=name)
            for et in range(DC):
                pst = ps_mm.tile([P, 512], fp32, tag="mm")
                for fc in range(DC):
                    nc.tensor.matmul(
                        pst,
                        lhsT=W2T[:, fc, et * P:(et + 1) * P],
                        rhs=wT[:, fc, :],
                        start=(fc == 0),
                        stop=(fc == DC - 1),
                    )
                nc.vector.tensor_copy(out=wf[:, et, :], in_=pst)
            return wf

        b2_bf = const.tile([P, DC], bf16, tag="b2bf")
        nc.vector.tensor_copy(out=b2_bf, in_=b2_sb)

        def head_bias(wT, name):
            kb = const.tile([P, NPAIR], fp32, tag=name)
            pst = ps_mm.tile([P, 512], fp32, tag="mm")
            for i in range(NPAIR):
                for dc in range(DC):
                    nc.tensor.matmul(
                        pst[:, i:i + 1],
                        lhsT=wT[:, dc, i * P:(i + 1) * P],
                        rhs=b2_bf[:, dc:dc + 1],
                        start=(i == 0 and dc == 0),
                        stop=(i == NPAIR - 1 and dc == DC - 1),
                    )
            nc.vector.tensor_copy(out=kb, in_=pst[:, 0:NPAIR])
            return kb

        Wfk = fuse_w2(WkT, "Wfk")
        Wfq = fuse_w2(WqT, "Wfq")
        kb = head_bias(WkT, "kb")
        qb = head_bias(WqT, "qb")

        # MLP stage-1 (relu) then fused head projection, per 512-token chunk
        def mlp_proj_chunk(xt, wf, bias_pair, dst, c):
                ht = hpool.tile([P, DC, 512], bf16, tag="hT")
                for t in range(DC):
                    pst = ps_mm.tile([P, 512], fp32, tag="mm")
                    nc.tensor.matmul(
                        pst,
                        lhsT=W1_bf[:, t * P:(t + 1) * P],
                        rhs=xt[:, c * 512:(c + 1) * 512],
                        start=True,
                        stop=True,
                    )
                    nc.vector.tensor_scalar(
                        ht[:, t, :], pst, b1_sb[:, t:t + 1], 0.0, ALU.add, ALU.max
                    )
                for i in range(NPAIR):
                    pst = ps_mm.tile([P, 512], fp32, tag="mm")
                    for e in range(DC):
                        nc.tensor.matmul(
                            pst,
                            lhsT=wf[:, e, i * P:(i + 1) * P],
                            rhs=ht[:, e, :],
                            start=(e == 0),
                            stop=(e == DC - 1),
                        )
                    nc.vector.tensor_scalar(
                        dst[:, i, c * 512:(c + 1) * 512], pst,
                        bias_pair[:, i:i + 1], None, ALU.add,
                    )

        def mlp_proj(xt, wf, bias_pair, dst):
            for c in range(MC):
                mlp_proj_chunk(xt, wf, bias_pair, dst, c)

        khT = seq.tile([P, NPAIR, N], bf16, tag="big16", bufs=3)
        mlp_proj(x1T, Wfk, kb, khT)
        qhT = seq.tile([P, NPAIR, N], bf16, tag="big16", bufs=3)
        mlp_proj_chunk(x2T, Wfq, qb, qhT, 0)

        # ---- r transpose + v projections + output-proj weights (deferred:
        # scores matmuls jump ahead of this on PE via high_priority) ----
        WvT = load_wT(Wv, "WvT")
        rT = seq.tile([P, DC, N], bf16, tag="big16", bufs=3)
        for t in range(NT):
            s = stage.tile([P, D], fp32, tag="stage")
            nc.sync.dma_start(out=s, in_=r_[t * P:(t + 1) * P, :])
            pst = ps_mm.tile([P, 512], fp32, tag="mm")
            for j in range(DC):
                nc.tensor.transpose(pst[:, j * P:(j + 1) * P], s[:, j * P:(j + 1) * P], ident)
            nc.vector.tensor_copy(
                out=rT[:, :, t * P:(t + 1) * P],
                in_=pst.rearrange("p (j q) -> p j q", j=DC),
            )

        vh = seq.tile([P, H, NT, HD + 1], bf16, tag="vh")
        nc.gpsimd.memset(vh[:, :, :, HD:HD + 1], 1.0)
        for t in range(NT):
            pst = ps_mm.tile([P, 512], fp32, tag="mm")
            for c in range(DC):
                nc.tensor.matmul(
                    pst,
                    lhsT=rT[:, c, t * P:(t + 1) * P],
                    rhs=WvT[:, c, :],
                    start=(c == 0),
                    stop=(c == DC - 1),
                )
            nc.vector.tensor_copy(
                out=vh[:, :, t, 0:HD], in_=pst.rearrange("p (h e) -> p h e", h=H)
            )

        # output proj, pair layout: WoT[s*64+e', pair, dv] = Wo[2*pair+s, dv, e']
        WoT = const.tile([P, NPAIR, D], bf16, tag="WoT")
        for i in range(NPAIR):
            ss = []
            for j in range(DC):
                sj = stage.tile([P, 2, HD], fp32, tag="wostage")
                nc.sync.dma_start(out=sj[:, 0, :], in_=Wo[2 * i, j * P:(j + 1) * P, :])
                nc.sync.dma_start(out=sj[:, 1, :], in_=Wo[2 * i + 1, j * P:(j + 1) * P, :])
                ss.append(sj)
            pst = ps_mm.tile([P, 512], fp32, tag="mm")
            for j in range(DC):
                nc.tensor.transpose(pst[:, j * P:(j + 1) * P], ss[j], ident)
            nc.vector.tensor_copy(out=WoT[:, i, :], in_=pst)

        # ---------------- attention ----------------
        # chunk-major so the per-chunk output projection overlaps the next
        # chunk's attention; PV iterates t-major so probs tiles release
        # incrementally and the next unit's scores can start early.
        o_hT = seq.tile([P, NPAIR, N], bf16, tag="big16", bufs=3)
        for c in range(MC):
            # just-in-time q-side MLP for this chunk: keeps these ~5us of PE
            # work out of the pre-vh serial bundle; they run in attention-
            # phase PE gaps while ScalarE digests the previous chunk.
            if c > 0:
                mlp_proj_chunk(x2T, Wfq, qb, qhT, c)
            for i in range(NPAIR):
                ptiles = []
                for t in range(NT):
                    ps = ps_sc.tile([P, 1024], fp32, tag="sc")
                    # High priority so the pair issues back-to-back on PE:
                    # the two matmuls occupy disjoint row groups (rows 0-63 /
                    # 64-127 via base_partition-derived tile_position) and run
                    # concurrently only if nothing lands between them.
                    with tc.high_priority(offset=8000):
                        nc.tensor.matmul(
                            ps[:, 0:512],
                            lhsT=khT[0:HD, i, t * P:(t + 1) * P],
                            rhs=qhT[0:HD, i, c * 512:(c + 1) * 512],
                            start=True,
                            stop=True,
                        )
                        nc.tensor.matmul(
                            ps[:, 512:1024],
                            lhsT=khT[HD:P, i, t * P:(t + 1) * P],
                            rhs=qhT[HD:P, i, c * 512:(c + 1) * 512],
                            start=True,
                            stop=True,
                        )
                    pt = probs.tile([P, 1024], bf16, tag="probs")
                    nc.scalar.activation(out=pt, in_=ps, func=AF.Exp, scale=0.125)
                    ptiles.append(pt)

                pos = [ps_po.tile([P, 4 * (HD + 1)], fp32, tag="po", name=f"po{si}")
                       for si in range(2)]
                # start=True clears has_written for the whole PSUM bank, so
                # only the tile's FIRST matmul may carry it; later regions'
                # first writes overwrite (cleared bits) then accumulate.
                for t in range(NT):
                    for si in range(2):
                        for mt in range(4):
                            nc.tensor.matmul(
                                pos[si][:, mt * (HD + 1):(mt + 1) * (HD + 1)],
                                lhsT=ptiles[t][:, si * 512 + mt * P: si * 512 + (mt + 1) * P],
                                rhs=vh[:, 2 * i + si, t, :],
                                start=(t == 0 and mt == 0),
                                stop=(t == NT - 1 and mt == 3),
                            )
                on = onorm.tile([P, 4, 2, HD], fp32, tag="onorm")
                for si in range(2):
                    po_v = pos[si].rearrange("p (mt e) -> p mt e", e=HD + 1)
                    rec = small.tile([P, 4], fp32, tag="rec")
                    nc.vector.reciprocal(rec, po_v[:, :, HD])
                    nc.vector.tensor_tensor(
                        out=on[:, :, si, :],
                        in0=po_v[:, :, 0:HD],
                        in1=rec[:, :, None].to_broadcast((P, 4, HD)),
                        op=ALU.mult,
                    )
                pst = ps_mm.tile([P, 512], fp32, tag="mm")
                for mt in range(4):
                    nc.tensor.transpose(pst[:, mt * P:(mt + 1) * P], on[:, mt, :, :], ident)
                nc.vector.tensor_copy(out=o_hT[:, i, c * 512:(c + 1) * 512], in_=pst)

            # ---- output projection for this chunk (sum over heads) ----
            # One K=128 matmul per head-pair: both operands stack the pair's
            # e' axes on partitions, and rep sums over heads, so contracting
            # all 128 partitions performs the head-pair sum in-instruction.
            for mt in range(4):
                t = c * 4 + mt
                psA = ps_mm.tile([P, 512], fp32, tag="mm")
                for i in range(NPAIR):
                    nc.tensor.matmul(
                        psA,
                        lhsT=o_hT[:, i, t * P:(t + 1) * P],
                        rhs=WoT[:, i, :],
                        start=(i == 0),
                        stop=(i == NPAIR - 1),
                    )
                ot = outp.tile([P, D], fp32, tag="out")
                nc.vector.tensor_copy(out=ot, in_=psA)
                nc.sync.dma_start(out=out[t * P:(t + 1) * P, :], in_=ot)

    nc.compile()
    return nc


def _get_program():
    if "nc" not in _CACHE:
        _CACHE["nc"] = _build_program()
    return _CACHE["nc"]


def kernel(x1, x2, r, W1, b1, W2, b2, Wq, Wk, Wv, Wo, trace=False):
    from concourse.bass_utils import run_bass_kernel_spmd

    nc = _get_program()

    def f32(a):
        return np.ascontiguousarray(np.asarray(a, dtype=np.float32))

    shared = {
        "W1": f32(W1), "b1": f32(b1), "W2": f32(W2), "b2": f32(b2),
        "Wq": f32(Wq), "Wk": f32(Wk), "Wv": f32(Wv), "Wo": f32(Wo),
    }
    in_maps = []
    for i in range(NCORES):
        m = dict(shared)
        m["x1"] = f32(x1[i])
        m["x2"] = f32(x2[i])
        m["r"] = f32(r[i])
        in_maps.append(m)

    res = run_bass_kernel_spmd(nc, in_maps, core_ids=list(range(NCORES)), trace=trace)
    out = np.stack([res.results[i]["out"] for i in range(NCORES)], axis=0)
    if trace:
        _CACHE["last_result"] = res
    return out


# revision 40
# speedup vs baseline: 1.0383x; 1.0383x over previous
"""Trainium2 Bass kernel for nn_Attention_28406913696361.

Architecture: B=8 batch elements -> 8 NeuronCores, pure data-parallel
(all params replicated, zero collectives). Each core computes, for its
batch element:
    k = mlp(x1), q = mlp(x2)          (shared 2-layer MLP, relu)
    qh/kh/vh = per-head projections    (H=8 heads, hd=64)
    o = softmax(qh kh^T / 8) vh        (full 2048x2048 attention)
    out = sum_h o_h @ Wo_h^T

Compute in bf16 with f32 PSUM accumulation (validated: L2 rel err ~5e-3
vs the f32 reference; gate is 2e-2). Key structural choices:

- W2 is folded into the per-head k/q projections (khT = (Wk W2) h^T), so
  MLP stage-2 and the 512-wide kq intermediate are never computed.
- Softmax denominators come free from a ones-column appended to vh: the
  PV matmul accumulates the row sum in f32 in the same instruction. No
  max-subtraction (scores/8 are bounded ~6), no vector reductions.
- exp runs on ScalarE (the bottleneck engine, ~1 elem/cycle/lane) from
  [128,1024] PSUM tiles; each tile holds one n-tile of scores for BOTH
  heads of a pair, computed by two matmuls on disjoint PE row groups
  (khT/qhT stack the pair's e' axes on partitions 0-63 / 64-127), which
  the hardware runs concurrently when issued back-to-back.
- Attention is chunk-major with the output projection woven in per
  chunk; PV iterates t-major so probs tiles release incrementally.
- Scores matmuls carry a large scheduler priority so they always beat
  queued PV matmuls: keeps ScalarE saturated.
- The output projection contracts all 128 partitions of the pair layout
  (o_hT pair-stacked e' x WoT pair-stacked e'), so one K=128 matmul per
  head-pair performs the head sum in-instruction.

Layouts (per core, partition dim first):
    x1T/x2T  [128, 2048]         d_x on partitions
    rT       [128, 4, 2048]      (d_inner, d_chunk, n)
    khT/qhT  [128, 4, 2048]      (s*64+e', head_pair, token)
    vh_ext   [128, 8, 16, 65]    (n_inner, head, n_tile, e'+ones)
    o_hT     [128, 4, 2048]      (s*64+e', pair, m)
"""

import numpy as np

N = 2048
DX = 128
D = 512
H = 8
HD = 64
P = 128
NT = N // P        # 16 token tiles
DC = D // P        # 4 feature chunks of 128
MC = N // 512      # 4 chunks of 512 tokens
NPAIR = H // 2     # 4 head pairs
NCORES = 8

_CACHE = {}


def _build_program():
    from contextlib import ExitStack

    import concourse.bass as bass  # noqa: F401
    import concourse.mybir as mybir
    import concourse.tile as tile
    from concourse import bacc
    from concourse.masks import make_identity

    fp32 = mybir.dt.float32
    bf16 = mybir.dt.bfloat16
    AF = mybir.ActivationFunctionType
    ALU = mybir.AluOpType

    nc = bacc.Bacc("TRN2")

    x1 = nc.declare_dram_parameter("x1", [N, DX], fp32, isOutput=False)
    x2 = nc.declare_dram_parameter("x2", [N, DX], fp32, isOutput=False)
    r_ = nc.declare_dram_parameter("r", [N, D], fp32, isOutput=False)
    W1 = nc.declare_dram_parameter("W1", [DX, D], fp32, isOutput=False)
    b1 = nc.declare_dram_parameter("b1", [D], fp32, isOutput=False)
    W2 = nc.declare_dram_parameter("W2", [D, D], fp32, isOutput=False)
    b2 = nc.declare_dram_parameter("b2", [D], fp32, isOutput=False)
    Wq = nc.declare_dram_parameter("Wq", [H, HD, D], fp32, isOutput=False)
    Wk = nc.declare_dram_parameter("Wk", [H, HD, D], fp32, isOutput=False)
    Wv = nc.declare_dram_parameter("Wv", [H, HD, D], fp32, isOutput=False)
    Wo = nc.declare_dram_parameter("Wo", [H, D, HD], fp32, isOutput=False)
    out = nc.declare_dram_parameter("out", [N, D], fp32, isOutput=True)

    with ExitStack() as ctx:
        tc = ctx.enter_context(tile.TileContext(nc))
        const = ctx.enter_context(tc.tile_pool(name="const", bufs=1))
        seq = ctx.enter_context(tc.tile_pool(name="seq", bufs=1))
        big = ctx.enter_context(tc.tile_pool(name="big", bufs=2))
        hpool = ctx.enter_context(tc.tile_pool(name="hpool", bufs=2))
        stage = ctx.enter_context(tc.tile_pool(name="stage", bufs=8))
        probs = ctx.enter_context(tc.tile_pool(name="probs", bufs=30))
        onorm = ctx.enter_context(tc.tile_pool(name="onorm", bufs=2))
        outp = ctx.enter_context(tc.tile_pool(name="outp", bufs=3))
        small = ctx.enter_context(tc.tile_pool(name="small", bufs=4))
        ps_mm = ctx.enter_context(tc.tile_pool(name="ps_mm", bufs=2, space="PSUM"))
        ps_sc = ctx.enter_context(tc.tile_pool(name="ps_sc", bufs=2, space="PSUM"))
        ps_po = ctx.enter_context(tc.tile_pool(name="ps_po", bufs=2, space="PSUM"))

        ident = const.tile([P, P], fp32, tag="ident")
        make_identity(nc, ident)

        # ---------------- weights ----------------
        s = stage.tile([P, D], fp32, tag="stage")
        nc.sync.dma_start(out=s, in_=W1[:, :])
        W1_bf = const.tile([P, D], bf16, tag="W1")
        nc.vector.tensor_copy(out=W1_bf, in_=s)

        w2stage = []
        for c in range(DC):
            s = stage.tile([P, D], fp32, tag="stage", name=f"w2s{c}")
            nc.sync.dma_start(out=s, in_=W2[c * P:(c + 1) * P, :])
            w2stage.append(s)

        b1_sb = const.tile([P, DC], fp32, tag="b1")
        b2_sb = const.tile([P, DC], fp32, tag="b2")
        with nc.allow_non_contiguous_dma(reason="tiny bias vectors"):
            nc.sync.dma_start(out=b1_sb, in_=b1.rearrange("(t p) -> p t", p=P))
            nc.sync.dma_start(out=b2_sb, in_=b2.rearrange("(t p) -> p t", p=P))

        # transposed qkv projection weights: [d_inner, d_chunk, (h e')]
        def load_wT(w_ap, name):
            wt = const.tile([P, DC, D], bf16, tag=name)
            flat = w_ap.rearrange("h e d -> (h e) d")
            ss = []
            for i in range(4):
                si = stage.tile([P, D], fp32, tag="stage")
                nc.sync.dma_start(out=si, in_=flat[i * P:(i + 1) * P, :])
                ss.append(si)
            for j in range(DC):
                pst = ps_mm.tile([P, 512], fp32, tag="mm")
                for i in range(4):
                    nc.tensor.transpose(
                        pst[:, i * P:(i + 1) * P], ss[i][:, j * P:(j + 1) * P], ident
                    )
                nc.vector.tensor_copy(out=wt[:, j, :], in_=pst)
            return wt

        # ---------------- input transposes ----------------
        def load_xT(x_ap):
            xt = big.tile([P, N], bf16, tag="xT")
            for g in range(4):
                ss = []
                for t in range(4):
                    si = stage.tile([P, P], fp32, tag="xstage")
                    tt = g * 4 + t
                    nc.sync.dma_start(out=si, in_=x_ap[tt * P:(tt + 1) * P, :])
                    ss.append(si)
                pst = ps_mm.tile([P, 512], fp32, tag="mm")
                for t in range(4):
                    nc.tensor.transpose(pst[:, t * P:(t + 1) * P], ss[t], ident)
                nc.vector.tensor_copy(out=xt[:, g * 512:(g + 1) * 512], in_=pst)
            return xt

        x1T = load_xT(x1)
        x2T = load_xT(x2)

        WkT = load_wT(Wk, "WkT")
        WqT = load_wT(Wq, "WqT")

        # ---- fuse W2 into the k/q head projections ----
        # khT = Wk (W2^T h^T) = (Wk W2) h^T, so precompute
        # Wfused[e, (h e')] = sum_f W2[e, f] WkT[f, (h e')] on PE (tiny), and
        # skip MLP stage-2 + the kq intermediate entirely. The b2 bias folds
        # to a per-(h,e') constant kb = Wk @ b2.
        W2T = const.tile([P, DC, D], bf16, tag="W2T")
        for j in range(DC):
            pst = ps_mm.tile([P, 512], fp32, tag="mm")
            for e in range(DC):
                nc.tensor.transpose(
                    pst[:, e * P:(e + 1) * P],
                    w2stage[e][:, j * P:(j + 1) * P], ident,
                )
            nc.vector.tensor_copy(out=W2T[:, j, :], in_=pst)

        def fuse_w2(wT, name):
            wf = const.tile([P, DC, D], bf16, tag=name)
            for et in range(DC):
                pst = ps_mm.tile([P, 512], fp32, tag="mm")
                for fc in range(DC):
                    nc.tensor.matmul(
                        pst,
                        lhsT=W2T[:, fc, et * P:(et + 1) * P],
                        rhs=wT[:, fc, :],
                        start=(fc == 0),
                        stop=(fc == DC - 1),
                    )
                nc.vector.tensor_copy(out=wf[:, et, :], in_=pst)
            return wf

        b2_bf = const.tile([P, DC], bf16, tag="b2bf")
        nc.vector.tensor_copy(out=b2_bf, in_=b2_sb)

        def head_bias(wT, name):
            kb = const.tile([P, NPAIR], fp32, tag=name)
            pst = ps_mm.tile([P, 512], fp32, tag="mm")
            for i in range(NPAIR):
                for dc in range(DC):
                    nc.tensor.matmul(
                        pst[:, i:i + 1],
                        lhsT=wT[:, dc, i * P:(i + 1) * P],
                        rhs=b2_bf[:, dc:dc + 1],
                        start=(i == 0 and dc == 0),
                        stop=(i == NPAIR - 1 and dc == DC - 1),
                    )
            nc.vector.tensor_copy(out=kb, in_=pst[:, 0:NPAIR])
            return kb

        Wfk = fuse_w2(WkT, "Wfk")
        Wfq = fuse_w2(WqT, "Wfq")
        kb = head_bias(WkT, "kb")
        qb = head_bias(WqT, "qb")

        # MLP stage-1 (relu) then fused head projection, per 512-token chunk
        def mlp_proj_chunk(xt, wf, bias_pair, dst, c):
                ht = hpool.tile([P, DC, 512], bf16, tag="hT")
                for t in range(DC):
                    pst = ps_mm.tile([P, 512], fp32, tag="mm")
                    nc.tensor.matmul(
                        pst,
                        lhsT=W1_bf[:, t * P:(t + 1) * P],
                        rhs=xt[:, c * 512:(c + 1) * 512],
                        start=True,
                        stop=True,
                    )
                    nc.vector.tensor_scalar(
                        ht[:, t, :], pst, b1_sb[:, t:t + 1], 0.0, ALU.add, ALU.max
                    )
                for i in range(NPAIR):
                    pst = ps_mm.tile([P, 512], fp32, tag="mm")
                    for e in range(DC):
                        nc.tensor.matmul(
                            pst,
                            lhsT=wf[:, e, i * P:(i + 1) * P],
                            rhs=ht[:, e, :],
                            start=(e == 0),
                            stop=(e == DC - 1),
                        )
                    nc.vector.tensor_scalar(
                        dst[:, i, c * 512:(c + 1) * 512], pst,
                        bias_pair[:, i:i + 1], None, ALU.add,
                    )

        def mlp_proj(xt, wf, bias_pair, dst):
            for c in range(MC):
                mlp_proj_chunk(xt, wf, bias_pair, dst, c)

        khT = seq.tile([P, NPAIR, N], bf16, tag="big16", bufs=3)
        mlp_proj(x1T, Wfk, kb, khT)
        qhT = seq.tile([P, NPAIR, N], bf16, tag="big16", bufs=3)
        mlp_proj(x2T, Wfq, qb, qhT)

        # ---- r transpose + v projections + output-proj weights (deferred:
        # scores matmuls jump ahead of this on PE via high_priority) ----
        WvT = load_wT(Wv, "WvT")
        rT = seq.tile([P, DC, N], bf16, tag="big16", bufs=3)
        for t in range(NT):
            s = stage.tile([P, D], fp32, tag="stage")
            nc.sync.dma_start(out=s, in_=r_[t * P:(t + 1) * P, :])
            pst = ps_mm.tile([P, 512], fp32, tag="mm")
            for j in range(DC):
                nc.tensor.transpose(pst[:, j * P:(j + 1) * P], s[:, j * P:(j + 1) * P], ident)
            nc.vector.tensor_copy(
                out=rT[:, :, t * P:(t + 1) * P],
                in_=pst.rearrange("p (j q) -> p j q", j=DC),
            )

        vh = seq.tile([P, H, NT, HD + 1], bf16, tag="vh")
        nc.gpsimd.memset(vh[:, :, :, HD:HD + 1], 1.0)
        for t in range(NT):
            pst = ps_mm.tile([P, 512], fp32, tag="mm")
            for c in range(DC):
                nc.tensor.matmul(
                    pst,
                    lhsT=rT[:, c, t * P:(t + 1) * P],
                    rhs=WvT[:, c, :],
                    start=(c == 0),
                    stop=(c == DC - 1),
                )
            nc.vector.tensor_copy(
                out=vh[:, :, t, 0:HD], in_=pst.rearrange("p (h e) -> p h e", h=H)
            )

        # output proj, pair layout: WoT[s*64+e', pair, dv] = Wo[2*pair+s, dv, e']
        WoT = const.tile([P, NPAIR, D], bf16, tag="WoT")
        for i in range(NPAIR):
            ss = []
            for j in range(DC):
                sj = stage.tile([P, 2, HD], fp32, tag="wostage")
                nc.sync.dma_start(out=sj[:, 0, :], in_=Wo[2 * i, j * P:(j + 1) * P, :])
                nc.sync.dma_start(out=sj[:, 1, :], in_=Wo[2 * i + 1, j * P:(j + 1) * P, :])
                ss.append(sj)
            pst = ps_mm.tile([P, 512], fp32, tag="mm")
            for j in range(DC):
                nc.tensor.transpose(pst[:, j * P:(j + 1) * P], ss[j], ident)
            nc.vector.tensor_copy(out=WoT[:, i, :], in_=pst)

        # ---------------- attention ----------------
        # chunk-major so the per-chunk output projection overlaps the next
        # chunk's attention; PV iterates t-major so probs tiles release
        # incrementally and the next unit's scores can start early.
        o_hT = seq.tile([P, NPAIR, N], bf16, tag="big16", bufs=3)
        for c in range(MC):
            for i in range(NPAIR):
                ptiles = []
                for t in range(NT):
                    ps = ps_sc.tile([P, 1024], fp32, tag="sc")
                    # High priority so the pair issues back-to-back on PE:
                    # the two matmuls occupy disjoint row groups (rows 0-63 /
                    # 64-127 via base_partition-derived tile_position) and run
                    # concurrently only if nothing lands between them.
                    with tc.high_priority(offset=8000):
                        nc.tensor.matmul(
                            ps[:, 0:512],
                            lhsT=khT[0:HD, i, t * P:(t + 1) * P],
                            rhs=qhT[0:HD, i, c * 512:(c + 1) * 512],
                            start=True,
                            stop=True,
                        )
                        nc.tensor.matmul(
                            ps[:, 512:1024],
                            lhsT=khT[HD:P, i, t * P:(t + 1) * P],
                            rhs=qhT[HD:P, i, c * 512:(c + 1) * 512],
                            start=True,
                            stop=True,
                        )
                    pt = probs.tile([P, 1024], bf16, tag="probs")
                    nc.scalar.activation(out=pt, in_=ps, func=AF.Exp, scale=0.125)
                    ptiles.append(pt)

                pos = [ps_po.tile([P, 4 * (HD + 1)], fp32, tag="po", name=f"po{si}")
                       for si in range(2)]
                # start=True clears has_written for the whole PSUM bank, so
                # only the tile's FIRST matmul may carry it; later regions'
                # first writes overwrite (cleared bits) then accumulate.
                for t in range(NT):
                    for si in range(2):
                        for mt in range(4):
                            nc.tensor.matmul(
                                pos[si][:, mt * (HD + 1):(mt + 1) * (HD + 1)],
                                lhsT=ptiles[t][:, si * 512 + mt * P: si * 512 + (mt + 1) * P],
                                rhs=vh[:, 2 * i + si, t, :],
                                start=(t == 0 and mt == 0),
                                stop=(t == NT - 1 and mt == 3),
                            )
                on = onorm.tile([P, 4, 2, HD], fp32, tag="onorm")
                for si in range(2):
                    po_v = pos[si].rearrange("p (mt e) -> p mt e", e=HD + 1)
                    rec = small.tile([P, 4], fp32, tag="rec")
                    nc.vector.reciprocal(rec, po_v[:, :, HD])
                    nc.vector.tensor_tensor(
                        out=on[:, :, si, :],
                        in0=po_v[:, :, 0:HD],
                        in1=rec[:, :, None].to_broadcast((P, 4, HD)),
                        op=ALU.mult,
                    )
                pst = ps_mm.tile([P, 512], fp32, tag="mm")
                for mt in range(4):
                    nc.tensor.transpose(pst[:, mt * P:(mt + 1) * P], on[:, mt, :, :], ident)
                nc.vector.tensor_copy(out=o_hT[:, i, c * 512:(c + 1) * 512], in_=pst)

            # ---- output projection for this chunk (sum over heads) ----
            # One K=128 matmul per head-pair: both operands stack the pair's
            # e' axes on partitions, and rep sums over heads, so contracting
            # all 128 partitions performs the head-pair sum in-instruction.
            for mt in range(4):
                t = c * 4 + mt
                psA = ps_mm.tile([P, 512], fp32, tag="mm")
                for i in range(NPAIR):
                    nc.tensor.matmul(
                        psA,
                        lhsT=o_hT[:, i, t * P:(t + 1) * P],
                        rhs=WoT[:, i, :],
                        start=(i == 0),
                        stop=(i == NPAIR - 1),
                    )
                ot = outp.tile([P, D], fp32, tag="out")
                nc.vector.tensor_copy(out=ot, in_=psA)
                nc.sync.dma_start(out=out[t * P:(t + 1) * P, :], in_=ot)

    nc.compile()
    return nc


def _get_program():
    if "nc" not in _CACHE:
        _CACHE["nc"] = _build_program()
    return _CACHE["nc"]


def kernel(x1, x2, r, W1, b1, W2, b2, Wq, Wk, Wv, Wo, trace=False):
    from concourse.bass_utils import run_bass_kernel_spmd

    nc = _get_program()

    def f32(a):
        return np.ascontiguousarray(np.asarray(a, dtype=np.float32))

    shared = {
        "W1": f32(W1), "b1": f32(b1), "W2": f32(W2), "b2": f32(b2),
        "Wq": f32(Wq), "Wk": f32(Wk), "Wv": f32(Wv), "Wo": f32(Wo),
    }
    in_maps = []
    for i in range(NCORES):
        m = dict(shared)
        m["x1"] = f32(x1[i])
        m["x2"] = f32(x2[i])
        m["r"] = f32(r[i])
        in_maps.append(m)

    res = run_bass_kernel_spmd(nc, in_maps, core_ids=list(range(NCORES)), trace=trace)
    out = np.stack([res.results[i]["out"] for i in range(NCORES)], axis=0)
    if trace:
        _CACHE["last_result"] = res
    return out


# revision 41
# speedup vs baseline: 1.0518x; 1.0130x over previous
"""Trainium2 Bass kernel for nn_Attention_28406913696361.

Architecture: B=8 batch elements -> 8 NeuronCores, pure data-parallel
(all params replicated, zero collectives). Each core computes, for its
batch element:
    k = mlp(x1), q = mlp(x2)          (shared 2-layer MLP, relu)
    qh/kh/vh = per-head projections    (H=8 heads, hd=64)
    o = softmax(qh kh^T / 8) vh        (full 2048x2048 attention)
    out = sum_h o_h @ Wo_h^T

Compute in bf16 with f32 PSUM accumulation (validated: L2 rel err ~5e-3
vs the f32 reference; gate is 2e-2). Key structural choices:

- W2 is folded into the per-head k/q projections (khT = (Wk W2) h^T), so
  MLP stage-2 and the 512-wide kq intermediate are never computed.
- Softmax denominators come free from a ones-column appended to vh: the
  PV matmul accumulates the row sum in f32 in the same instruction. No
  max-subtraction (scores/8 are bounded ~6), no vector reductions.
- exp runs on ScalarE (the bottleneck engine, ~1 elem/cycle/lane) from
  [128,1024] PSUM tiles; each tile holds one n-tile of scores for BOTH
  heads of a pair, computed by two matmuls on disjoint PE row groups
  (khT/qhT stack the pair's e' axes on partitions 0-63 / 64-127), which
  the hardware runs concurrently when issued back-to-back.
- Attention is chunk-major with the output projection woven in per
  chunk; PV iterates t-major so probs tiles release incrementally.
- Scores matmuls carry a large scheduler priority so they always beat
  queued PV matmuls: keeps ScalarE saturated.
- The output projection contracts all 128 partitions of the pair layout
  (o_hT pair-stacked e' x WoT pair-stacked e'), so one K=128 matmul per
  head-pair performs the head sum in-instruction.

Layouts (per core, partition dim first):
    x1T/x2T  [128, 2048]         d_x on partitions
    rT       [128, 4, 2048]      (d_inner, d_chunk, n)
    khT/qhT  [128, 4, 2048]      (s*64+e', head_pair, token)
    vh_ext   [128, 8, 16, 65]    (n_inner, head, n_tile, e'+ones)
    o_hT     [128, 4, 2048]      (s*64+e', pair, m)
"""

import numpy as np

N = 2048
DX = 128
D = 512
H = 8
HD = 64
P = 128
NT = N // P        # 16 token tiles
DC = D // P        # 4 feature chunks of 128
MC = N // 512      # 4 chunks of 512 tokens
NPAIR = H // 2     # 4 head pairs
NCORES = 8

_CACHE = {}


def _build_program():
    from contextlib import ExitStack

    import concourse.bass as bass  # noqa: F401
    import concourse.mybir as mybir
    import concourse.tile as tile
    from concourse import bacc
    from concourse.masks import make_identity

    fp32 = mybir.dt.float32
    bf16 = mybir.dt.bfloat16
    AF = mybir.ActivationFunctionType
    ALU = mybir.AluOpType

    nc = bacc.Bacc("TRN2")

    x1 = nc.declare_dram_parameter("x1", [N, DX], fp32, isOutput=False)
    x2 = nc.declare_dram_parameter("x2", [N, DX], fp32, isOutput=False)
    r_ = nc.declare_dram_parameter("r", [N, D], fp32, isOutput=False)
    W1 = nc.declare_dram_parameter("W1", [DX, D], fp32, isOutput=False)
    b1 = nc.declare_dram_parameter("b1", [D], fp32, isOutput=False)
    W2 = nc.declare_dram_parameter("W2", [D, D], fp32, isOutput=False)
    b2 = nc.declare_dram_parameter("b2", [D], fp32, isOutput=False)
    Wq = nc.declare_dram_parameter("Wq", [H, HD, D], fp32, isOutput=False)
    Wk = nc.declare_dram_parameter("Wk", [H, HD, D], fp32, isOutput=False)
    Wv = nc.declare_dram_parameter("Wv", [H, HD, D], fp32, isOutput=False)
    Wo = nc.declare_dram_parameter("Wo", [H, D, HD], fp32, isOutput=False)
    out = nc.declare_dram_parameter("out", [N, D], fp32, isOutput=True)

    with ExitStack() as ctx:
        tc = ctx.enter_context(tile.TileContext(nc))
        const = ctx.enter_context(tc.tile_pool(name="const", bufs=1))
        seq = ctx.enter_context(tc.tile_pool(name="seq", bufs=1))
        big = ctx.enter_context(tc.tile_pool(name="big", bufs=2))
        hpool = ctx.enter_context(tc.tile_pool(name="hpool", bufs=2))
        stage = ctx.enter_context(tc.tile_pool(name="stage", bufs=8))
        probs = ctx.enter_context(tc.tile_pool(name="probs", bufs=30))
        onorm = ctx.enter_context(tc.tile_pool(name="onorm", bufs=2))
        outp = ctx.enter_context(tc.tile_pool(name="outp", bufs=3))
        small = ctx.enter_context(tc.tile_pool(name="small", bufs=4))
        ps_mm = ctx.enter_context(tc.tile_pool(name="ps_mm", bufs=2, space="PSUM"))
        ps_sc = ctx.enter_context(tc.tile_pool(name="ps_sc", bufs=2, space="PSUM"))
        ps_po = ctx.enter_context(tc.tile_pool(name="ps_po", bufs=2, space="PSUM"))

        ident = const.tile([P, P], fp32, tag="ident")
        make_identity(nc, ident)

        # ---------------- weights ----------------
        s = stage.tile([P, D], fp32, tag="stage")
        nc.sync.dma_start(out=s, in_=W1[:, :])
        W1_bf = const.tile([P, D], bf16, tag="W1")
        nc.vector.tensor_copy(out=W1_bf, in_=s)

        w2stage = []
        for c in range(DC):
            s = stage.tile([P, D], fp32, tag="stage", name=f"w2s{c}")
            nc.sync.dma_start(out=s, in_=W2[c * P:(c + 1) * P, :])
            w2stage.append(s)

        b1_sb = const.tile([P, DC], fp32, tag="b1")
        b2_sb = const.tile([P, DC], fp32, tag="b2")
        with nc.allow_non_contiguous_dma(reason="tiny bias vectors"):
            nc.sync.dma_start(out=b1_sb, in_=b1.rearrange("(t p) -> p t", p=P))
            nc.sync.dma_start(out=b2_sb, in_=b2.rearrange("(t p) -> p t", p=P))

        # transposed qkv projection weights: [d_inner, d_chunk, (h e')]
        def load_wT(w_ap, name):
            wt = const.tile([P, DC, D], bf16, tag=name)
            flat = w_ap.rearrange("h e d -> (h e) d")
            ss = []
            for i in range(4):
                si = stage.tile([P, D], fp32, tag="stage")
                nc.sync.dma_start(out=si, in_=flat[i * P:(i + 1) * P, :])
                ss.append(si)
            for j in range(DC):
                pst = ps_mm.tile([P, 512], fp32, tag="mm")
                for i in range(4):
                    nc.tensor.transpose(
                        pst[:, i * P:(i + 1) * P], ss[i][:, j * P:(j + 1) * P], ident
                    )
                nc.vector.tensor_copy(out=wt[:, j, :], in_=pst)
            return wt

        # ---------------- input transposes ----------------
        def load_xT(x_ap):
            xt = big.tile([P, N], bf16, tag="xT")
            for g in range(4):
                ss = []
                for t in range(4):
                    si = stage.tile([P, P], fp32, tag="xstage")
                    tt = g * 4 + t
                    nc.sync.dma_start(out=si, in_=x_ap[tt * P:(tt + 1) * P, :])
                    ss.append(si)
                pst = ps_mm.tile([P, 512], fp32, tag="mm")
                for t in range(4):
                    nc.tensor.transpose(pst[:, t * P:(t + 1) * P], ss[t], ident)
                nc.vector.tensor_copy(out=xt[:, g * 512:(g + 1) * 512], in_=pst)
            return xt

        x1T = load_xT(x1)
        x2T = load_xT(x2)

        WkT = load_wT(Wk, "WkT")
        WqT = load_wT(Wq, "WqT")

        # ---- fuse W2 into the k/q head projections ----
        # khT = Wk (W2^T h^T) = (Wk W2) h^T, so precompute
        # Wfused[e, (h e')] = sum_f W2[e, f] WkT[f, (h e')] on PE (tiny), and
        # skip MLP stage-2 + the kq intermediate entirely. The b2 bias folds
        # to a per-(h,e') constant kb = Wk @ b2.
        W2T = const.tile([P, DC, D], bf16, tag="W2T")
        for j in range(DC):
            pst = ps_mm.tile([P, 512], fp32, tag="mm")
            for e in range(DC):
                nc.tensor.transpose(
                    pst[:, e * P:(e + 1) * P],
                    w2stage[e][:, j * P:(j + 1) * P], ident,
                )
            nc.vector.tensor_copy(out=W2T[:, j, :], in_=pst)

        def fuse_w2(wT, name):
            wf = const.tile([P, DC, D], bf16, tag=name)
            for et in range(DC):
                pst = ps_mm.tile([P, 512], fp32, tag="mm")
                for fc in range(DC):
                    nc.tensor.matmul(
                        pst,
                        lhsT=W2T[:, fc, et * P:(et + 1) * P],
                        rhs=wT[:, fc, :],
                        start=(fc == 0),
                        stop=(fc == DC - 1),
                    )
                nc.vector.tensor_copy(out=wf[:, et, :], in_=pst)
            return wf

        b2_bf = const.tile([P, DC], bf16, tag="b2bf")
        nc.vector.tensor_copy(out=b2_bf, in_=b2_sb)

        def head_bias(wT, name):
            kb = const.tile([P, NPAIR], fp32, tag=name)
            pst = ps_mm.tile([P, 512], fp32, tag="mm")
            for i in range(NPAIR):
                for dc in range(DC):
                    nc.tensor.matmul(
                        pst[:, i:i + 1],
                        lhsT=wT[:, dc, i * P:(i + 1) * P],
                        rhs=b2_bf[:, dc:dc + 1],
                        start=(i == 0 and dc == 0),
                        stop=(i == NPAIR - 1 and dc == DC - 1),
                    )
            nc.vector.tensor_copy(out=kb, in_=pst[:, 0:NPAIR])
            return kb

        Wfk = fuse_w2(WkT, "Wfk")
        Wfq = fuse_w2(WqT, "Wfq")
        kb = head_bias(WkT, "kb")
        qb = head_bias(WqT, "qb")

        # MLP stage-1 (relu) then fused head projection, per 512-token chunk
        def mlp_proj_chunk(xt, wf, bias_pair, dst, c):
                ht = hpool.tile([P, DC, 512], bf16, tag="hT")
                for t in range(DC):
                    pst = ps_mm.tile([P, 512], fp32, tag="mm")
                    nc.tensor.matmul(
                        pst,
                        lhsT=W1_bf[:, t * P:(t + 1) * P],
                        rhs=xt[:, c * 512:(c + 1) * 512],
                        start=True,
                        stop=True,
                    )
                    nc.vector.tensor_scalar(
                        ht[:, t, :], pst, b1_sb[:, t:t + 1], 0.0, ALU.add, ALU.max
                    )
                for i in range(NPAIR):
                    pst = ps_mm.tile([P, 512], fp32, tag="mm")
                    for e in range(DC):
                        nc.tensor.matmul(
                            pst,
                            lhsT=wf[:, e, i * P:(i + 1) * P],
                            rhs=ht[:, e, :],
                            start=(e == 0),
                            stop=(e == DC - 1),
                        )
                    nc.vector.tensor_scalar(
                        dst[:, i, c * 512:(c + 1) * 512], pst,
                        bias_pair[:, i:i + 1], None, ALU.add,
                    )

        def mlp_proj(xt, wf, bias_pair, dst):
            for c in range(MC):
                mlp_proj_chunk(xt, wf, bias_pair, dst, c)

        khT = seq.tile([P, NPAIR, N], bf16, tag="big16", bufs=3)
        mlp_proj(x1T, Wfk, kb, khT)
        qhT = seq.tile([P, NPAIR, N], bf16, tag="big16", bufs=3)
        mlp_proj(x2T, Wfq, qb, qhT)

        # ---- r transpose + v projections + output-proj weights (deferred:
        # scores matmuls jump ahead of this on PE via high_priority) ----
        WvT = load_wT(Wv, "WvT")
        rT = seq.tile([P, DC, N], bf16, tag="big16", bufs=3)
        for t in range(NT):
            s = stage.tile([P, D], fp32, tag="stage")
            nc.sync.dma_start(out=s, in_=r_[t * P:(t + 1) * P, :])
            pst = ps_mm.tile([P, 512], fp32, tag="mm")
            for j in range(DC):
                nc.tensor.transpose(pst[:, j * P:(j + 1) * P], s[:, j * P:(j + 1) * P], ident)
            nc.vector.tensor_copy(
                out=rT[:, :, t * P:(t + 1) * P],
                in_=pst.rearrange("p (j q) -> p j q", j=DC),
            )

        vh = seq.tile([P, H, NT, HD + 1], bf16, tag="vh")
        nc.gpsimd.memset(vh[:, :, :, HD:HD + 1], 1.0)
        for t in range(NT):
            pst = ps_mm.tile([P, 512], fp32, tag="mm")
            for c in range(DC):
                nc.tensor.matmul(
                    pst,
                    lhsT=rT[:, c, t * P:(t + 1) * P],
                    rhs=WvT[:, c, :],
                    start=(c == 0),
                    stop=(c == DC - 1),
                )
            nc.vector.tensor_copy(
                out=vh[:, :, t, 0:HD], in_=pst.rearrange("p (h e) -> p h e", h=H)
            )

        # output proj, pair layout: WoT[s*64+e', pair, dv] = Wo[2*pair+s, dv, e']
        WoT = const.tile([P, NPAIR, D], bf16, tag="WoT")
        for i in range(NPAIR):
            ss = []
            for j in range(DC):
                sj = stage.tile([P, 2, HD], fp32, tag="wostage")
                nc.sync.dma_start(out=sj[:, 0, :], in_=Wo[2 * i, j * P:(j + 1) * P, :])
                nc.sync.dma_start(out=sj[:, 1, :], in_=Wo[2 * i + 1, j * P:(j + 1) * P, :])
                ss.append(sj)
            pst = ps_mm.tile([P, 512], fp32, tag="mm")
            for j in range(DC):
                nc.tensor.transpose(pst[:, j * P:(j + 1) * P], ss[j], ident)
            nc.vector.tensor_copy(out=WoT[:, i, :], in_=pst)

        # ---------------- attention ----------------
        # chunk-major so the per-chunk output projection overlaps the next
        # chunk's attention; PV iterates t-major so probs tiles release
        # incrementally and the next unit's scores can start early.
        o_hT = seq.tile([P, NPAIR, N], bf16, tag="big16", bufs=3)
        for c in range(MC):
            for i in range(NPAIR):
                ptiles = []
                for t in range(NT):
                    ps = ps_sc.tile([P, 1024], fp32, tag="sc")
                    # High priority so the pair issues back-to-back on PE:
                    # the two matmuls occupy disjoint row groups (rows 0-63 /
                    # 64-127 via base_partition-derived tile_position) and run
                    # concurrently only if nothing lands between them.
                    with tc.high_priority(offset=8000):
                        nc.tensor.matmul(
                            ps[:, 0:512],
                            lhsT=khT[0:HD, i, t * P:(t + 1) * P],
                            rhs=qhT[0:HD, i, c * 512:(c + 1) * 512],
                            start=True,
                            stop=True,
                        )
                        nc.tensor.matmul(
                            ps[:, 512:1024],
                            lhsT=khT[HD:P, i, t * P:(t + 1) * P],
                            rhs=qhT[HD:P, i, c * 512:(c + 1) * 512],
                            start=True,
                            stop=True,
                        )
                    pt = probs.tile([P, 1024], bf16, tag="probs")
                    nc.scalar.activation(out=pt, in_=ps, func=AF.Exp, scale=0.125)
                    ptiles.append(pt)

                pos = [ps_po.tile([P, 4 * (HD + 1)], fp32, tag="po", name=f"po{si}")
                       for si in range(2)]
                # start=True clears has_written for the whole PSUM bank, so
                # only the tile's FIRST matmul may carry it; later regions'
                # first writes overwrite (cleared bits) then accumulate.
                # Mid-tier priority (above the deferred rT/vh background,
                # below scores): lets each PV t-step preempt the stream as
                # soon as its vh[h,t] lands, releasing probs tiles so exp
                # trickles through the vh-production window instead of
                # stalling until vh fully completes.
                with tc.high_priority(offset=4000):
                    for t in range(NT):
                        for si in range(2):
                            for mt in range(4):
                                nc.tensor.matmul(
                                    pos[si][:, mt * (HD + 1):(mt + 1) * (HD + 1)],
                                    lhsT=ptiles[t][:, si * 512 + mt * P: si * 512 + (mt + 1) * P],
                                    rhs=vh[:, 2 * i + si, t, :],
                                    start=(t == 0 and mt == 0),
                                    stop=(t == NT - 1 and mt == 3),
                                )
                on = onorm.tile([P, 4, 2, HD], fp32, tag="onorm")
                for si in range(2):
                    po_v = pos[si].rearrange("p (mt e) -> p mt e", e=HD + 1)
                    rec = small.tile([P, 4], fp32, tag="rec")
                    nc.vector.reciprocal(rec, po_v[:, :, HD])
                    nc.vector.tensor_tensor(
                        out=on[:, :, si, :],
                        in0=po_v[:, :, 0:HD],
                        in1=rec[:, :, None].to_broadcast((P, 4, HD)),
                        op=ALU.mult,
                    )
                pst = ps_mm.tile([P, 512], fp32, tag="mm")
                for mt in range(4):
                    nc.tensor.transpose(pst[:, mt * P:(mt + 1) * P], on[:, mt, :, :], ident)
                nc.vector.tensor_copy(out=o_hT[:, i, c * 512:(c + 1) * 512], in_=pst)

            # ---- output projection for this chunk (sum over heads) ----
            # One K=128 matmul per head-pair: both operands stack the pair's
            # e' axes on partitions, and rep sums over heads, so contracting
            # all 128 partitions performs the head-pair sum in-instruction.
            for mt in range(4):
                t = c * 4 + mt
                psA = ps_mm.tile([P, 512], fp32, tag="mm")
                for i in range(NPAIR):
                    nc.tensor.matmul(
                        psA,
                        lhsT=o_hT[:, i, t * P:(t + 1) * P],
                        rhs=WoT[:, i, :],
                        start=(i == 0),
                        stop=(i == NPAIR - 1),
                    )
                ot = outp.tile([P, D], fp32, tag="out")
                nc.vector.tensor_copy(out=ot, in_=psA)
                nc.sync.dma_start(out=out[t * P:(t + 1) * P, :], in_=ot)

    nc.compile()
    return nc


def _get_program():
    if "nc" not in _CACHE:
        _CACHE["nc"] = _build_program()
    return _CACHE["nc"]


def kernel(x1, x2, r, W1, b1, W2, b2, Wq, Wk, Wv, Wo, trace=False):
    from concourse.bass_utils import run_bass_kernel_spmd

    nc = _get_program()

    def f32(a):
        return np.ascontiguousarray(np.asarray(a, dtype=np.float32))

    shared = {
        "W1": f32(W1), "b1": f32(b1), "W2": f32(W2), "b2": f32(b2),
        "Wq": f32(Wq), "Wk": f32(Wk), "Wv": f32(Wv), "Wo": f32(Wo),
    }
    in_maps = []
    for i in range(NCORES):
        m = dict(shared)
        m["x1"] = f32(x1[i])
        m["x2"] = f32(x2[i])
        m["r"] = f32(r[i])
        in_maps.append(m)

    res = run_bass_kernel_spmd(nc, in_maps, core_ids=list(range(NCORES)), trace=trace)
    out = np.stack([res.results[i]["out"] for i in range(NCORES)], axis=0)
    if trace:
        _CACHE["last_result"] = res
    return out
